# revision 1
# baseline (speedup 1.0000x reference)
"""v15: v12 + per-head bf16 staging (hbf) in the outer pool scope so its
alloc barrier clears at t=0 and the Pool-queue reads run as soon as the
scratch data exists (they previously waited ~90us for phase-A SBUF to free).
Paid for by moving kT to the bf16 DRAM scratch (net -4KB SBUF). wo loads
emit after head 0's reads.

v12: v9 + two-sweep v-projection: sweep 1 accumulates 8 tk tiles across
all 8 PSUM banks chunk-by-chunk as x arrives (PE saturated during the x
load); sweep 2 finishes tk 8-15 once x is resident.

v9: v8 but per-head attention reads are normal phase-B loads on the idle
Pool queue (v8 emitted them in phase A with a single-buffered pool that
parked the SP queue head on a long semaphore wait - and hit
NRT_EXEC_UNIT_UNRECOVERABLE on hardware).

Original: Trainium2 Bass kernel for LGeM self-attention (b=2, t=2048, c=2048, h=16, d=128).

v8: fp32r matmuls (self-loading weights; bf16 matmuls pay a ~550ns/mm
ldweights penalty on real TRN2; walrus rejects mixed-dtype matmuls and
ldw-opt). All host->device tensors ship bf16; upcasts to fp32r run on DVE
(8-deep exec queue) and ACT, which are idle during the projections.

Schedule/queue design (from timeline-sim gap analysis):
  - pools open before any emission so no engine's first DMA waits on pool
    allocation barriers; wv chunk loads (Pool/SWDGE queue) are emitted
    before the x loads so the first v-proj matmul starts ~3us in,
  - x chunks upcast through per-chunk fp32r tiles (dependency granularity
    = one chunk, not the whole 128KB tile),
  - wq/wk slices load on the ACT queue (idle after the x odd-chunk loads);
    Pool/SWDGE is slow (~1us/DMA engine time) so it only carries wv/wo,
  - qT and v round-trip a bf16 DRAM scratch on the SP queue; the per-head
    attention reads are emitted inside phase A right after their head's
    scratch writes, so they land long before attention needs them,
  - kT stays SBUF-resident bf16; cf factors ship bf16 and feed mixed-dtype
    DVE multiplies (f32 PSUM x bf16 -> bf16),
  - output is written bf16; host sums the mp-partials in fp32.

Numerics: the validated "bf16 everywhere" dataflow (5e-3 max rel err vs fp32
reference; tolerance 2e-2) with fp32r accumulation between stages.

Sharding: 8 cores = 2 (batch) x 4 (head-groups of 4 heads). Per core: q/k/v
projections for its 4 heads, attention, partial output projection (its 512
rows of Wo); host sums the 4 mp-partials per batch.

Math notes (matching the reference exactly):
  - rope is q*(cos+sin) elementwise (the module's rotate_half is identity),
    with the 1/sqrt(t) logit scale folded into cfq.
  - softmax without max-subtraction: logits ~N(0,0.2^2), exp cannot overflow.
  - scores are built transposed, S_T[tk,tq] = kT_tile.T @ qT, so attn@v needs
    no transposes; the softmax denominator comes from a ones[128,128]
    stationary matmul (free partition-broadcast for the reciprocal multiply).
"""

import sys

sys.path.insert(0, "/opt/trn_rl_repo")

import math

import numpy as np
import ml_dtypes

import concourse.bass as bass
import concourse.mybir as mybir
import concourse.tile as tile
from concourse import bacc, bass_utils

F32 = mybir.dt.float32
F32R = mybir.dt.float32r
BF16 = mybir.dt.bfloat16
NP_BF16 = ml_dtypes.bfloat16
Copy = mybir.ActivationFunctionType.Copy
Exp = mybir.ActivationFunctionType.Exp

HIDDEN = 2048
HEADS = 16
HEAD_DIM = 128
SEQ = 2048
BATCH = 2
N_CORES = 8
MP = 4
HG = HEADS // MP
THETA = 10000.0


def build_attention_nc(T, C, HG, D, use_mask=False):
    DG = HG * D  # 512
    CCH = C // 128  # 16
    TQC = min(512, T)
    NTQ = T // TQC  # 4
    NTK = T // 128  # 16
    NQT = T // 128  # 16
    NOC = C // TQC  # 4

    nc = bacc.Bacc("TRN2", target_bir_lowering=False, debug=False)

    xT = nc.dram_tensor("xT", [C, T], BF16, kind="ExternalInput").ap()
    wq = nc.dram_tensor("wq", [C, DG], BF16, kind="ExternalInput").ap()
    wk = nc.dram_tensor("wk", [C, DG], BF16, kind="ExternalInput").ap()
    wv = nc.dram_tensor("wv", [C, DG], BF16, kind="ExternalInput").ap()
    wo = nc.dram_tensor("wo", [DG, C], BF16, kind="ExternalInput").ap()
    cfq = nc.dram_tensor("cfq", [D, T], BF16, kind="ExternalInput").ap()
    cfk = nc.dram_tensor("cfk", [D, T], BF16, kind="ExternalInput").ap()
    if use_mask:
        maskT = nc.dram_tensor("maskT", [T, T], F32, kind="ExternalInput").ap()
    out = nc.dram_tensor("out", [T, C], BF16, kind="ExternalOutput").ap()

    with tile.TileContext(nc) as tc:
        with (
            tc.tile_pool(name="scratch", bufs=1, space="DRAM") as dpool,
            tc.tile_pool(name="hbf", bufs=1) as hbfpool,
        ):
            qT_s = dpool.tile([DG, T], BF16, tag="qTs")
            kT_s = dpool.tile([DG, T], BF16, tag="kTs")
            v_s = dpool.tile([T, DG], BF16, tag="vs")

            # ---------------- Phase A ----------------
            with tc.tile_pool(name="xp", bufs=1) as xpool:
                xT_c = [
                    xpool.tile([128, T], F32R, name=f"xc{cc}", tag=f"xc{cc}")
                    for cc in range(CCH)
                ]
                with tc.tile_pool(name="cf", bufs=1) as cfpool:
                    cfq_sb = cfpool.tile([128, T], BF16, tag="cfq")
                    cfk_sb = cfpool.tile([128, T], BF16, tag="cfk")

                    with (
                        tc.tile_pool(name="xbf", bufs=3) as xbfpool,
                        tc.tile_pool(name="wvbf", bufs=1) as wvbfpool,
                        tc.tile_pool(name="wvf", bufs=1) as wvfpool,
                        tc.tile_pool(name="vst", bufs=2) as vstpool,
                        tc.tile_pool(name="vps", bufs=1, space="PSUM") as vps,
                    ):
                        # wv first on the Pool queue: first chunks land ~2us
                        wv_c = [
                            wvfpool.tile(
                                [128, DG], F32R, name=f"wvc{cc}", tag=f"wvc{cc}"
                            )
                            for cc in range(CCH)
                        ]
                        for cc in range(CCH):
                            wvb = wvbfpool.tile([128, DG], BF16, tag="wvb")
                            nc.gpsimd.dma_start(
                                wvb[:], wv[cc * 128 : (cc + 1) * 128, :]
                            )
                            if cc % 2 == 0:
                                nc.vector.tensor_copy(wv_c[cc][:], wvb[:])
                            else:
                                nc.scalar.activation(wv_c[cc][:], wvb[:], Copy)
                        for cc in range(CCH):
                            xb = xbfpool.tile([128, T], BF16, tag="xb")
                            eng = nc.sync if cc % 2 == 0 else nc.scalar
                            eng.dma_start(xb[:], xT[cc * 128 : (cc + 1) * 128, :])
                            if cc % 2 == 0:
                                nc.vector.tensor_copy(xT_c[cc][:], xb[:])
                            else:
                                nc.scalar.activation(xT_c[cc][:], xb[:], Copy)
                        # cf (bf16) after the x chunks on the SP queue
                        nc.sync.dma_start(cfq_sb[:D, :], cfq)
                        nc.sync.dma_start(cfk_sb[:D, :], cfk)
                        # v-proj sweep 1: 8 tk tiles accumulate in parallel,
                        # chunk-major, so PE has 8 matmuls of work per arriving
                        # x chunk instead of stalling on the full contraction.
                        NSW = 8
                        pvs = [
                            vps.tile([128, DG], F32, name=f"pv{tk}", tag=f"pv{tk}")
                            for tk in range(NSW)
                        ]
                        for cc in range(CCH):
                            for tk in range(NSW):
                                nc.tensor.matmul(
                                    pvs[tk][:],
                                    xT_c[cc][:, tk * 128 : (tk + 1) * 128],
                                    wv_c[cc][:],
                                    start=(cc == 0),
                                    stop=(cc == CCH - 1),
                                )
                        for tk in range(NSW):
                            vt = vstpool.tile([128, DG], BF16, tag="vt")
                            nc.vector.tensor_copy(vt[:], pvs[tk][:])
                            nc.sync.dma_start(v_s[tk * 128 : (tk + 1) * 128, :], vt[:])
                        # sweep 2: remaining tk at full speed (x resident)
                        for tk in range(NSW, NTK):
                            pv = vps.tile([128, DG], F32, name=f"pv{(tk - NSW) % NSW}", tag=f"pv{(tk - NSW) % NSW}")
                            for cc in range(CCH):
                                nc.tensor.matmul(
                                    pv[:],
                                    xT_c[cc][:, tk * 128 : (tk + 1) * 128],
                                    wv_c[cc][:],
                                    start=(cc == 0),
                                    stop=(cc == CCH - 1),
                                )
                            vt = vstpool.tile([128, DG], BF16, tag="vt")
                            nc.vector.tensor_copy(vt[:], pv[:])
                            nc.sync.dma_start(v_s[tk * 128 : (tk + 1) * 128, :], vt[:])

                    # ---- qT/kT ----
                    with (
                        tc.tile_pool(name="wbf", bufs=3) as wbfpool,
                        tc.tile_pool(name="wcc", bufs=2) as wccpool,
                        tc.tile_pool(name="qst", bufs=2) as qstpool,
                        tc.tile_pool(name="qkps", bufs=4, space="PSUM") as qkps,
                    ):
                        for h in range(HG):
                            for wi, (w_in, cf_sb) in enumerate(
                                ((wq, cfq_sb), (wk, cfk_sb))
                            ):
                                wb = wbfpool.tile([128, CCH, D], BF16, tag="wb")
                                for cc in range(CCH):
                                    nc.scalar.dma_start(
                                        wb[:, cc, :],
                                        w_in[
                                            cc * 128 : (cc + 1) * 128,
                                            h * D : (h + 1) * D,
                                        ],
                                    )
                                wc = wccpool.tile([128, CCH, D], F32R, tag="wc")
                                nc.vector.tensor_copy(wc[:], wb[:])
                                for tq in range(NTQ):
                                    pm = qkps.tile([128, TQC], F32)
                                    for cc in range(CCH):
                                        nc.tensor.matmul(
                                            pm[:],
                                            wc[:, cc, :],
                                            xT_c[cc][:, tq * TQC : (tq + 1) * TQC],
                                            start=(cc == 0),
                                            stop=(cc == CCH - 1),
                                        )
                                    dst_s = qT_s if wi == 0 else kT_s
                                    qs = qstpool.tile([128, TQC], BF16, tag="qs")
                                    nc.vector.tensor_mul(
                                        qs[:D, :],
                                        pm[:D, :],
                                        cf_sb[:D, tq * TQC : (tq + 1) * TQC],
                                    )
                                    nc.sync.dma_start(
                                        dst_s[
                                            h * D : (h + 1) * D,
                                            tq * TQC : (tq + 1) * TQC,
                                        ],
                                        qs[:D, :],
                                    )

            # ---------------- Phase B: attention ----------------
            with tc.tile_pool(name="o2", bufs=1) as o2pool:
                out2_sb = o2pool.tile([128, HG, T], F32R)
                with (
                    tc.tile_pool(name="wobf", bufs=2) as wobfpool,
                    tc.tile_pool(name="wop", bufs=1) as wopool,
                ):
                    with (
                        tc.tile_pool(name="hp", bufs=2) as hpool,
                        tc.tile_pool(name="cst", bufs=1) as cstpool,
                        tc.tile_pool(name="ep", bufs=6) as epool,
                        tc.tile_pool(name="mp", bufs=4) as mpool,
                        tc.tile_pool(name="rp", bufs=2) as rpool,
                        tc.tile_pool(name="scps", bufs=4, space="PSUM") as scps,
                        tc.tile_pool(name="o2ps", bufs=2, space="PSUM") as o2ps,
                        tc.tile_pool(name="sps", bufs=2, space="PSUM") as sps,
                    ):
                        ones_f = cstpool.tile([128, 128], F32)
                        nc.vector.memset(ones_f[:], 1.0)
                        ones = cstpool.tile([128, 128], F32R)
                        nc.vector.tensor_copy(ones[:], ones_f[:])
                        wo_sb = wopool.tile([128, HG, C], F32R)

                        def _emit_wo_loads():
                            for hh in range(HG):
                                wob = wobfpool.tile([128, C], BF16, tag="wob")
                                nc.gpsimd.dma_start(
                                    wob[:D, :], wo[hh * D : (hh + 1) * D, :]
                                )
                                if hh % 2 == 0:
                                    nc.vector.tensor_copy(
                                        wo_sb[:D, hh, :], wob[:D, :]
                                    )
                                else:
                                    nc.scalar.activation(
                                        wo_sb[:D, hh, :], wob[:D, :], Copy
                                    )
                        for h in range(HG):
                            if h == 1:
                                _emit_wo_loads()
                            qT_sb = hpool.tile([128, T], F32R, tag="qT")
                            kT_sb = hpool.tile([128, T], F32R, tag="kT")
                            v_sb = hpool.tile([128, NTK, D], F32R, tag="v")
                            qbf = hbfpool.tile([128, T], BF16, tag="qbf")
                            kbf = hbfpool.tile([128, T], BF16, tag="kbf")
                            vbf = hbfpool.tile([128, NTK, D], BF16, tag="vbf")
                            nc.gpsimd.dma_start(qbf[:D, :], qT_s[h * D : (h + 1) * D, :])
                            nc.gpsimd.dma_start(kbf[:D, :], kT_s[h * D : (h + 1) * D, :])
                            for i in range(NTK):
                                nc.gpsimd.dma_start(
                                    vbf[:, i, :],
                                    v_s[i * 128 : (i + 1) * 128, h * D : (h + 1) * D],
                                )
                            nc.vector.tensor_copy(kT_sb[:D, :], kbf[:D, :])
                            nc.vector.tensor_copy(qT_sb[:D, :], qbf[:D, :])
                            nc.vector.tensor_copy(v_sb[:, :, :], vbf[:, :, :])
                            for tq in range(NTQ):
                                o2p = o2ps.tile([128, TQC], F32)
                                sp = sps.tile([128, TQC], F32)
                                pending = []
                                for tk in range(NTK):
                                    scp = scps.tile([128, TQC], F32)
                                    nc.tensor.matmul(
                                        scp[:],
                                        kT_sb[:D, tk * 128 : (tk + 1) * 128],
                                        qT_sb[:D, tq * TQC : (tq + 1) * TQC],
                                        start=True,
                                        stop=True,
                                    )
                                    et = epool.tile([128, TQC], F32R, tag="et")
                                    if use_mask:
                                        mt = mpool.tile([128, TQC], F32, tag="mt")
                                        nc.gpsimd.dma_start(
                                            mt[:],
                                            maskT[
                                                tk * 128 : (tk + 1) * 128,
                                                tq * TQC : (tq + 1) * TQC,
                                            ],
                                        )
                                        ma = mpool.tile([128, TQC], F32, tag="ma")
                                        nc.vector.tensor_add(ma[:], scp[:], mt[:])
                                        nc.scalar.activation(et[:], ma[:], Exp)
                                    else:
                                        nc.scalar.activation(et[:], scp[:], Exp)
                                    pending.append((et, tk))
                                    if len(pending) > 2:
                                        p_et, p_tk = pending.pop(0)
                                        nc.tensor.matmul(
                                            o2p[:],
                                            v_sb[:, p_tk, :],
                                            p_et[:],
                                            start=(p_tk == 0),
                                            stop=False,
                                        )
                                        nc.tensor.matmul(
                                            sp[:],
                                            ones[:],
                                            p_et[:],
                                            start=(p_tk == 0),
                                            stop=False,
                                        )
                                while pending:
                                    p_et, p_tk = pending.pop(0)
                                    nc.tensor.matmul(
                                        o2p[:],
                                        v_sb[:, p_tk, :],
                                        p_et[:],
                                        start=(p_tk == 0),
                                        stop=(p_tk == NTK - 1),
                                    )
                                    nc.tensor.matmul(
                                        sp[:],
                                        ones[:],
                                        p_et[:],
                                        start=(p_tk == 0),
                                        stop=(p_tk == NTK - 1),
                                    )
                                rt = rpool.tile([128, TQC], F32)
                                nc.vector.reciprocal(rt[:], sp[:])
                                nc.vector.tensor_mul(
                                    out2_sb[:D, h, tq * TQC : (tq + 1) * TQC],
                                    o2p[:D, :],
                                    rt[:D, :],
                                )

                    # -------- output projection --------
                    with (
                        tc.tile_pool(name="fst", bufs=4) as fpool,
                        tc.tile_pool(name="fps", bufs=4, space="PSUM") as fps,
                    ):
                        for qt in range(NQT):
                            for oc in range(NOC):
                                fp = fps.tile([128, TQC], F32)
                                for h in range(HG):
                                    nc.tensor.matmul(
                                        fp[:],
                                        out2_sb[:D, h, qt * 128 : (qt + 1) * 128],
                                        wo_sb[:D, h, oc * TQC : (oc + 1) * TQC],
                                        start=(h == 0),
                                        stop=(h == HG - 1),
                                    )
                                ft = fpool.tile([128, TQC], BF16, tag="ft")
                                if oc % 2 == 0:
                                    nc.vector.tensor_copy(ft[:], fp[:])
                                else:
                                    nc.scalar.activation(ft[:], fp[:], Copy)
                                nc.sync.dma_start(
                                    out[
                                        qt * 128 : (qt + 1) * 128,
                                        oc * TQC : (oc + 1) * TQC,
                                    ],
                                    ft[:],
                                )

    nc.compile()
    return nc


def compute_cfacs(T, D, theta=THETA):
    freq = 1.0 / theta ** (np.arange(0, D, 2, dtype=np.float64) / D)
    t = np.arange(T, dtype=np.float64)
    m = np.einsum("i,j->ij", t, freq)
    m = np.concatenate([m, m], axis=-1)
    cfac = (np.cos(m) + np.sin(m)).astype(np.float32)
    cfk = np.ascontiguousarray(cfac.T).astype(NP_BF16)
    cfq = np.ascontiguousarray(cfac.T / np.float32(math.sqrt(T))).astype(NP_BF16)
    return cfq, cfk


_NC_CACHE = {}


def _get_nc(use_mask):
    key = bool(use_mask)
    if key not in _NC_CACHE:
        _NC_CACHE[key] = build_attention_nc(SEQ, HIDDEN, HG, HEAD_DIM, use_mask=key)
    return _NC_CACHE[key]


def _make_in_maps(input_ids, Wq, Wk, Wv, Wo, attention_mask=None):
    DG = HG * HEAD_DIM
    cfq, cfk = compute_cfacs(SEQ, HEAD_DIM)
    xb = [np.ascontiguousarray(input_ids[bi].T).astype(NP_BF16) for bi in range(BATCH)]
    wqb = Wq.astype(NP_BF16)
    wkb = Wk.astype(NP_BF16)
    wvb = Wv.astype(NP_BF16)
    wob = Wo.astype(NP_BF16)
    in_maps = []
    for core in range(N_CORES):
        bi, g = divmod(core, MP)
        m = {
            "xT": xb[bi],
            "wq": np.ascontiguousarray(wqb[:, g * DG : (g + 1) * DG]),
            "wk": np.ascontiguousarray(wkb[:, g * DG : (g + 1) * DG]),
            "wv": np.ascontiguousarray(wvb[:, g * DG : (g + 1) * DG]),
            "wo": np.ascontiguousarray(wob[g * DG : (g + 1) * DG, :]),
            "cfq": cfq,
            "cfk": cfk,
        }
        if attention_mask is not None:
            m["maskT"] = np.ascontiguousarray(attention_mask[bi, 0].T)
        in_maps.append(m)
    return in_maps


def prepare_for_bench(inputs):
    input_ids = np.asarray(inputs["input_ids"], dtype=np.float32)
    Wq = np.asarray(inputs["Wq"], dtype=np.float32)
    Wk = np.asarray(inputs["Wk"], dtype=np.float32)
    Wv = np.asarray(inputs["Wv"], dtype=np.float32)
    Wo = np.asarray(inputs["Wo"], dtype=np.float32)
    return _get_nc(False), _make_in_maps(input_ids, Wq, Wk, Wv, Wo)


def kernel(input_ids, attention_mask, Wq, Wk, Wv, Wo):
    input_ids = np.asarray(input_ids, dtype=np.float32)
    attention_mask = np.asarray(attention_mask, dtype=np.float32)
    Wq = np.asarray(Wq, dtype=np.float32)
    Wk = np.asarray(Wk, dtype=np.float32)
    Wv = np.asarray(Wv, dtype=np.float32)
    Wo = np.asarray(Wo, dtype=np.float32)

    b, t, c = input_ids.shape
    assert (b, t, c) == (BATCH, SEQ, HIDDEN)

    use_mask = bool(np.any(attention_mask))
    nc = _get_nc(use_mask)
    in_maps = _make_in_maps(
        input_ids, Wq, Wk, Wv, Wo, attention_mask if use_mask else None
    )

    res = bass_utils.run_bass_kernel_spmd(nc, in_maps, core_ids=list(range(N_CORES)))

    out = np.zeros((BATCH, SEQ, HIDDEN), dtype=np.float32)
    for bi in range(BATCH):
        acc = res.results[bi * MP]["out"].astype(np.float32)
        for g in range(1, MP):
            acc = acc + res.results[bi * MP + g]["out"].astype(np.float32)
        out[bi] = acc
    return out



# revision 14
# speedup vs baseline: 1.6327x; 1.6327x over previous
"""v17: fp8-DoubleRow rewrite of the LGeM self-attention kernel.

Key facts (from the instruction cost model, which is the graded metric):
  - fp8(e4m3) matmuls with MatmulPerfMode.DoubleRow cost 0.5 cycles per
    OUTPUT column and contract 2 k-subtiles per instruction -> 4x the
    fp32r/bf16 rate for a fixed contraction.
  - bf16 and fp16 matmuls cost the same (1.0 cyc/row), so the non-fp8
    pipeline (v, et, out2, Wo) runs in fp16 for 8x less rounding error.
  - ACT exp costs 0.833ns/elem regardless of dtype (+~185ns/inst): exp over
    [128, 2, 512] PSUM groups amortizes it; ACT is exp-only in phase B.
  - DVE fp16 elementwise ops hit the 2x perf mode (~376ns per [128,512]).
  - All DMA transfers serialize on one shared device at ~360GB/s, in issue
    order -- so loads are issued in dependency-criticality order.

Design (per core: one batch element, 4 heads = 512 of 2048 Wq/Wk/Wv cols
and 512 rows of Wo; host sums the 4 tensor-parallel partials per batch):
  - Everything SBUF-resident; no DRAM scratch round-trips.
  - Phase A in three PSUM scopes so no accumulation group is left open
    waiting on late DMA data:
      A1: v-proj chain 1 (x_hi @ Wv_hi, fp8 DR) -> v_sb fp16 (holds 64*v;
          the 64 is divided out for free via ones=64 in the denominator).
      A2: q/k-proj (fp8 DR from x_hi) -> DVE rope-mul (cos+sin factors with
          all fp8 scale folding) -> fp8 q/k; SBUF->SBUF DMAs rearrange
          [128d, T] into the [64, 2, T] d-split DoubleRow layout.
      A3: v-proj correction chains (x_lo @ Wv_hi + x_hi @ Wv_lo, fp8 DR)
          added into v_sb by DVE (compensates fp8 quantization of x and Wv).
  - Phase B per tq-512 block, per head: scores fp8 DR over d=[64,2]; exp on
    ACT (scale=1/32 descales fp8) over 2-bank PSUM groups -> fp16 et;
    attn@v fp16; denominator = DVE/Pool fp16 tree over the 16 et tiles +
    one ones[128,128]@acc matmul (replaces 16 PE matmuls per (h,tq)).
  - out-proj fp16, emitted through a queue that interleaves its (qt,oc)
    groups between attention exp-groups of the NEXT tq block, so PE never
    stalls on the single out-proj PSUM bank; the final block's groups
    alternate between two banks (reusing the idle sp bank).

Numerics: fp8 quantization residuals are compensated where errors pass
straight through (v), and exploited where softmax attenuates them
(q/k/scores: fp8 noise lands on logits ~N(0,0.2^2) as ~1.5% attention
weight noise -> ~1.3e-2 worst-case rel err vs the 2e-2 gate).
"""

import sys

sys.path.insert(0, "/opt/trn_rl_repo")

import math

import numpy as np
import ml_dtypes

import concourse.bass as bass
import concourse.mybir as mybir
import concourse.tile as tile
from concourse import bacc, bass_utils

F32 = mybir.dt.float32
FP16 = mybir.dt.float16
FP8 = mybir.dt.float8e4
NP_FP8 = ml_dtypes.float8_e4m3
Copy = mybir.ActivationFunctionType.Copy
Exp = mybir.ActivationFunctionType.Exp
DR = mybir.MatmulPerfMode.DoubleRow

HIDDEN = 2048
HEADS = 16
HEAD_DIM = 128
SEQ = 2048
BATCH = 2
N_CORES = 8
MP = 4
HG = HEADS // MP  # 4 heads per core
THETA = 10000.0

WSCALE = 64.0  # host premultiplier on Wq/Wk/Wv before fp8 cast
QSCALE = 32.0  # fp8 scale on roped q (folded into cfq with the 1/WSCALE)


def build_attention_nc(use_mask=False):
    T, C, D, DG = SEQ, HIDDEN, HEAD_DIM, HG * HEAD_DIM
    CCH = C // 128  # 16 contraction subtiles
    CCP = CCH // 2  # 8 DoubleRow pairs
    NTK = T // 128  # 16
    NTQ = T // 512  # 4 tq blocks
    NG = NTK // 2  # 8 exp groups per (h, tq)

    nc = bacc.Bacc("TRN2", target_bir_lowering=False, debug=False)

    x8hi = nc.dram_tensor("x8hi", [C, T], FP8, kind="ExternalInput").ap()
    x8lo = nc.dram_tensor("x8lo", [C, T], FP8, kind="ExternalInput").ap()
    wv8hi = nc.dram_tensor("wv8hi", [C, DG], FP8, kind="ExternalInput").ap()
    wv8lo = nc.dram_tensor("wv8lo", [C, DG], FP8, kind="ExternalInput").ap()
    w8q = nc.dram_tensor("w8q", [C, DG], FP8, kind="ExternalInput").ap()
    w8k = nc.dram_tensor("w8k", [C, DG], FP8, kind="ExternalInput").ap()
    wo = nc.dram_tensor("wo", [HG, D, C], FP16, kind="ExternalInput").ap()
    cfq = nc.dram_tensor("cfq", [D, T], FP16, kind="ExternalInput").ap()
    cfk = nc.dram_tensor("cfk", [D, T], FP16, kind="ExternalInput").ap()
    if use_mask:
        maskT = nc.dram_tensor("maskT", [T, T], F32, kind="ExternalInput").ap()
    out = nc.dram_tensor("out", [T, C], FP16, kind="ExternalOutput").ap()

    with tile.TileContext(nc) as tc:
        with (
            tc.tile_pool(name="res", bufs=1) as res,
            tc.tile_pool(name="accp", bufs=2) as accp,
            tc.tile_pool(name="o2sb", bufs=2) as o2sb,
            tc.tile_pool(name="rtp", bufs=2) as rtp,
            tc.tile_pool(name="otp", bufs=4) as otp,
            tc.tile_pool(name="mkp", bufs=2) as mkp,
        ):
            wo_s = res.tile([128, HG, C], FP16, tag="wo_s")
            cfq_s = res.tile([128, T], FP16, tag="cfq_s")
            cfk_s = res.tile([128, T], FP16, tag="cfk_s")
            v_sb = res.tile([128, NTK, DG], FP16, tag="v_sb")
            qT16 = res.tile([128, HG, T], FP16, tag="qT16")
            kT16 = res.tile([128, HG, T], FP16, tag="kT16")
            ones = res.tile([128, 128], FP16, tag="ones")

            # ones = 64 folds the 1/WSCALE of v_sb (which holds 64*v) into
            # the softmax denominator reciprocal.
            nc.vector.memset(ones[:], WSCALE)

            # ---- phase A (x/w staging SBUF is scoped so phase B can hold
            # a full slot of et tiles) ----
            with (
                tc.tile_pool(name="xw", bufs=1) as xw,
                tc.tile_pool(name="aps", bufs=1, space="PSUM") as aps,
            ):
                x8hi_s = xw.tile([128, CCH, T], FP8, tag="x8hi_s")
                x8lo_s = xw.tile([128, CCH, T], FP8, tag="x8lo_s")
                wv8hi_s = xw.tile([128, CCH, DG], FP8, tag="wv8hi_s")
                wv8lo_s = xw.tile([128, CCH, DG], FP8, tag="wv8lo_s")
                w8q_s = xw.tile([128, CCH, DG], FP8, tag="w8q_s")
                w8k_s = xw.tile([128, CCH, DG], FP8, tag="w8k_s")

                # Input loads: ALL on one queue (SP) because transfers from
                # different queues round-robin on the shared DMA device --
                # one queue enforces global dependency-criticality order.
                # wv8hi chunk-pairs interleave with x8hi pairs so each
                # DoubleRow ccp step's operands arrive together.
                wvh_r = wv8hi.rearrange("(a p) b -> p a b", p=128)
                wvl_r = wv8lo.rearrange("(a p) b -> p a b", p=128)
                for ccp in range(CCP):
                    nc.sync.dma_start(
                        wv8hi_s[:, 2 * ccp : 2 * ccp + 2, :],
                        wvh_r[:, 2 * ccp : 2 * ccp + 2, :],
                    )
                    for cc in (2 * ccp, 2 * ccp + 1):
                        nc.sync.dma_start(
                            x8hi_s[:, cc, :], x8hi[cc * 128 : (cc + 1) * 128, :]
                        )
                nc.sync.dma_start(
                    w8q_s[:, :, :], w8q.rearrange("(a p) b -> p a b", p=128)
                )
                nc.sync.dma_start(
                    w8k_s[:, :, :], w8k.rearrange("(a p) b -> p a b", p=128)
                )
                nc.sync.dma_start(cfq_s[:D, :], cfq)
                nc.sync.dma_start(cfk_s[:D, :], cfk)
                nc.sync.dma_start(wv8lo_s[:, :, :], wvl_r)
                for cc in range(CCH):
                    nc.sync.dma_start(
                        x8lo_s[:, cc, :], x8lo[cc * 128 : (cc + 1) * 128, :]
                    )
                for hh in range(HG):
                    nc.sync.dma_start(wo_s[:D, hh, :], wo[hh])

                # A1/A3 rounds and A2's qk tiles all rotate through one
                # 8-bank tag space -- no PSUM scope transitions in phase A.
                def vchain(round_tks, chains, vop):
                    pvs = [
                        aps.tile([128, DG], F32, tag=f"pv{i}", name=f"pv{i}")
                        for i in range(len(round_tks))
                    ]
                    n = len(chains) * CCP
                    k = 0
                    for ccp in range(CCP):
                        for x_s, w_s in chains:
                            for i, tk in enumerate(round_tks):
                                nc.tensor.matmul(
                                    pvs[i][:],
                                    x_s[:, 2 * ccp : 2 * ccp + 2, tk * 128 : (tk + 1) * 128],
                                    w_s[:, 2 * ccp : 2 * ccp + 2, :],
                                    start=(k == 0),
                                    stop=(k == n - 1),
                                    perf_mode=DR,
                                )
                            k += 1
                    for i, tk in enumerate(round_tks):
                        vop(tk, pvs[i])

                # A1: v-proj chain 1 (x_hi @ Wv_hi) -> v_sb (holds 64*v)
                for r in range(2):
                    vchain(
                        list(range(r * 8, r * 8 + 8)),
                        [(x8hi_s, wv8hi_s)],
                        lambda tk, pv: nc.vector.tensor_copy(v_sb[:, tk, :], pv[:]),
                    )

                # A2: q/k projection -> rope-mul -> fp8 -> d-split rearrange
                qpi = [0]
                for h in range(HG):
                    for wi, (w_s, cf_s, dst) in enumerate(
                        ((w8q_s, cfq_s, qT16), (w8k_s, cfk_s, kT16))
                    ):
                        for tq in range(NTQ):
                            qp = aps.tile(
                                [128, 512], F32, tag=f"pv{qpi[0] % 8}", name="qp"
                            )
                            qpi[0] += 1
                            for ccp in range(CCP):
                                nc.tensor.matmul(
                                    qp[:],
                                    w_s[:, 2 * ccp : 2 * ccp + 2, h * D : (h + 1) * D],
                                    x8hi_s[:, 2 * ccp : 2 * ccp + 2, tq * 512 : (tq + 1) * 512],
                                    start=(ccp == 0),
                                    stop=(ccp == CCP - 1),
                                    perf_mode=DR,
                                )
                            nc.vector.tensor_mul(
                                dst[:D, h, tq * 512 : (tq + 1) * 512],
                                qp[:D, :],
                                cf_s[:D, tq * 512 : (tq + 1) * 512],
                            )

                # A3: v-proj correction chains (x_lo@Wv_hi + x_hi@Wv_lo)
                for r in range(2):
                    vchain(
                        list(range(r * 8, r * 8 + 8)),
                        [(x8lo_s, wv8hi_s), (x8hi_s, wv8lo_s)],
                        lambda tk, pv: nc.vector.tensor_add(
                            v_sb[:, tk, :], v_sb[:, tk, :], pv[:]
                        ),
                    )

            # ---- phase B: slot-pipelined attention + out-projection ----
            # Slot k = (tq, h). PE emits scores(k) interleaved with
            # attn@v(k-1) -- a FULL slot of lag, so attn@v never waits on
            # ACT's exp stream. Finisher(k-1) (denominator matmul, recip,
            # normalize) and out-proj drips fill the remaining PE slack.
            with (
                tc.tile_pool(name="etp", bufs=1) as etp,
                tc.tile_pool(name="scps", bufs=1, space="PSUM") as scps,
                tc.tile_pool(name="o2ps", bufs=1, space="PSUM") as o2ps,
                tc.tile_pool(name="mps", bufs=1, space="PSUM") as mps,
            ):
                og_queue = []
                og_i = [0]
                fin_queue = []
                TAGROT = ["fp", "sp"]

                def emit_og(tags=("fp", "sp")):
                    tag = tags[og_i[0] % len(tags)]
                    og_i[0] += 1
                    qt, oc, o2t, tq = og_queue.pop(0)
                    fp = mps.tile([128, 512], F32, tag=tag, name="fp") if tag in (
                        "fp",
                        "sp",
                    ) else o2ps.tile([128, 512], F32, tag=tag, name="fp")
                    for hh in range(HG):
                        nc.tensor.matmul(
                            fp[:],
                            o2t[:D, hh, qt * 128 : (qt + 1) * 128],
                            wo_s[:D, hh, oc * 512 : (oc + 1) * 512],
                            start=(hh == 0),
                            stop=(hh == HG - 1),
                        )
                    ot = otp.tile([128, 512], FP16, tag="ot")
                    if og_i[0] % 2 == 0:
                        nc.scalar.activation(ot[:], fp[:], Copy)
                    else:
                        nc.vector.tensor_copy(ot[:], fp[:])
                    nc.sync.dma_start(
                        out[
                            tq * 512 + qt * 128 : tq * 512 + (qt + 1) * 128,
                            oc * 512 : (oc + 1) * 512,
                        ],
                        ot[:],
                    )

                def flush_fin():
                    # out-proj groups for a tq block enter the queue HERE,
                    # once its last head is normalized -- dripping them any
                    # earlier would read o2t[h3] before it is written.
                    acc, o2p, o2t, h, tq = fin_queue.pop(0)
                    spp = mps.tile([128, 512], F32, tag="sp", name="spp")
                    nc.tensor.matmul(spp[:], ones[:], acc[:], start=True, stop=True)
                    rt = rtp.tile([128, 512], F32, tag="rt")
                    nc.vector.reciprocal(rt[:], spp[:])
                    nc.vector.tensor_mul(o2t[:D, h, :], o2p[:D, :], rt[:D, :])
                    if h == HG - 1:
                        for qt in range(4):
                            for oc in range(4):
                                og_queue.append((qt, oc, o2t, tq))

                slots = [(tq, h) for tq in range(NTQ) for h in range(HG)]
                prev = None  # (ets, o2p, o2t, h, acc) of slot k-1

                def emit_av(prev, g):
                    ets, o2p, o2t, h, acc, _tq = prev
                    et = ets[g]
                    for j in range(2):
                        tk = 2 * g + j
                        nc.tensor.matmul(
                            o2p[:],
                            v_sb[:, tk, h * D : (h + 1) * D],
                            et[:, j, :],
                            start=(tk == 0),
                            stop=(tk == NTK - 1),
                        )
                    eng = nc.vector if g % 2 == 0 else nc.gpsimd
                    eng.tensor_add(et[:, 0, :], et[:, 0, :], et[:, 1, :])
                    if g == 1:
                        nc.vector.tensor_add(acc[:], ets[0][:, 0, :], ets[1][:, 0, :])
                    elif g > 1:
                        nc.vector.tensor_add(acc[:], acc[:], et[:, 0, :])

                for k, (tq, h) in enumerate(slots):
                    if h == 0:
                        o2t = o2sb.tile([128, HG, 512], FP16, tag="o2", name="o2t")
                    o2p = o2ps.tile([128, 512], F32, tag=f"o2p{k % 2}", name="o2p")
                    acc = accp.tile([128, 512], FP16, tag="acc")
                    ets = []
                    for g in range(NG):
                        sc = scps.tile(
                            [128, 2, 512], F32, tag=f"sc{g % 2}", name="sc"
                        )
                        for j in range(2):
                            tk = 2 * g + j
                            nc.tensor.matmul(
                                sc[:, j, :],
                                kT16[:D, h, tk * 128 : (tk + 1) * 128],
                                qT16[:D, h, tq * 512 : (tq + 1) * 512],
                                start=True,
                                stop=True,
                            )
                        if use_mask:
                            mt = mkp.tile([128, 2, 512], F32, tag="mt")
                            for j in range(2):
                                nc.gpsimd.dma_start(
                                    mt[:, j, :],
                                    maskT[
                                        (2 * g + j) * 128 : (2 * g + j + 1) * 128,
                                        tq * 512 : (tq + 1) * 512,
                                    ],
                                )
                            nc.vector.tensor_add(sc[:, :, :], sc[:, :, :], mt[:])
                        et = etp.tile([128, 2, 512], FP16, tag=f"et{(k % 2) * 8 + g}")
                        nc.scalar.activation(et[:], sc[:], Exp)
                        ets.append(et)
                        # previous slot's attn@v + denominator tree
                        if prev is not None:
                            emit_av(prev, g)
                        if g == 2 and fin_queue:
                            flush_fin()
                        if g in (3, 5, 7) and og_queue:
                            emit_og()
                            if g == 7 and og_queue and len(og_queue) % 4 == 1:
                                emit_og()
                    if prev is not None:
                        fin_queue.append((prev[4], prev[1], prev[2], prev[3], prev[5]))
                    prev = (ets, o2p, o2t, h, acc, tq)
                # drain: last slot's attn@v, remaining finishers, then the
                # final out-proj groups rotating over four idle banks
                for g in range(NG):
                    emit_av(prev, g)
                    if g in (3, 5, 7) and og_queue:
                        emit_og()
                fin_queue.append((prev[4], prev[1], prev[2], prev[3], prev[5]))
                while fin_queue:
                    flush_fin()
                og_i[0] = 0
                while og_queue:
                    emit_og(("fp", "o2p0", "sp", "o2p1"))

    nc.compile()
    return nc


def compute_cfacs():
    T, D = SEQ, HEAD_DIM
    freq = 1.0 / THETA ** (np.arange(0, D, 2, dtype=np.float64) / D)
    t = np.arange(T, dtype=np.float64)
    m = np.einsum("i,j->ij", t, freq)
    m = np.concatenate([m, m], axis=-1)
    cfac = (np.cos(m) + np.sin(m)).T  # [D, T]
    cfq = (cfac / math.sqrt(T) / WSCALE).astype(np.float16)
    cfk = (cfac / WSCALE).astype(np.float16)
    return cfq, cfk


_NC_CACHE = {}


def _get_nc(use_mask):
    key = bool(use_mask)
    if key not in _NC_CACHE:
        _NC_CACHE[key] = build_attention_nc(use_mask=key)
    return _NC_CACHE[key]


def _split_fp8(a):
    hi = a.astype(NP_FP8)
    lo = (a - hi.astype(np.float32)).astype(NP_FP8)
    return hi, lo


def _make_in_maps(input_ids, Wq, Wk, Wv, Wo, attention_mask=None):
    DG = HG * HEAD_DIM
    cfq, cfk = compute_cfacs()
    xhi, xlo = [], []
    for bi in range(BATCH):
        xT = np.ascontiguousarray(input_ids[bi].T)
        hi, lo = _split_fp8(xT)
        xhi.append(hi)
        xlo.append(lo)
    in_maps = []
    for core in range(N_CORES):
        bi, g = divmod(core, MP)
        sl = slice(g * DG, (g + 1) * DG)
        wvhi, wvlo = _split_fp8(np.ascontiguousarray(Wv[:, sl]) * WSCALE)
        m = {
            "x8hi": xhi[bi],
            "x8lo": xlo[bi],
            "wv8hi": wvhi,
            "wv8lo": wvlo,
            "w8q": (np.ascontiguousarray(Wq[:, sl]) * WSCALE).astype(NP_FP8),
            "w8k": (np.ascontiguousarray(Wk[:, sl]) * WSCALE).astype(NP_FP8),
            "wo": np.ascontiguousarray(Wo[sl, :])
            .reshape(HG, HEAD_DIM, HIDDEN)
            .astype(np.float16),
            "cfq": cfq,
            "cfk": cfk,
        }
        if attention_mask is not None:
            m["maskT"] = np.ascontiguousarray(attention_mask[bi, 0].T).astype(
                np.float32
            )
        in_maps.append(m)
    return in_maps


def prepare_for_bench(inputs):
    input_ids = np.asarray(inputs["input_ids"], dtype=np.float32)
    Wq = np.asarray(inputs["Wq"], dtype=np.float32)
    Wk = np.asarray(inputs["Wk"], dtype=np.float32)
    Wv = np.asarray(inputs["Wv"], dtype=np.float32)
    Wo = np.asarray(inputs["Wo"], dtype=np.float32)
    return _get_nc(False), _make_in_maps(input_ids, Wq, Wk, Wv, Wo)


def kernel(input_ids, attention_mask, Wq, Wk, Wv, Wo):
    input_ids = np.asarray(input_ids, dtype=np.float32)
    attention_mask = np.asarray(attention_mask, dtype=np.float32)
    Wq = np.asarray(Wq, dtype=np.float32)
    Wk = np.asarray(Wk, dtype=np.float32)
    Wv = np.asarray(Wv, dtype=np.float32)
    Wo = np.asarray(Wo, dtype=np.float32)

    b, t, c = input_ids.shape
    assert (b, t, c) == (BATCH, SEQ, HIDDEN)

    use_mask = bool(np.any(attention_mask))
    nc = _get_nc(use_mask)
    in_maps = _make_in_maps(
        input_ids, Wq, Wk, Wv, Wo, attention_mask if use_mask else None
    )

    res = bass_utils.run_bass_kernel_spmd(nc, in_maps, core_ids=list(range(N_CORES)))

    out = np.zeros((BATCH, SEQ, HIDDEN), dtype=np.float32)
    for bi in range(BATCH):
        acc = res.results[bi * MP]["out"].astype(np.float32)
        for g in range(1, MP):
            acc = acc + res.results[bi * MP + g]["out"].astype(np.float32)
        out[bi] = acc
    return out


# revision 16
# speedup vs baseline: 1.6623x; 1.0181x over previous
"""v17: fp8-DoubleRow rewrite of the LGeM self-attention kernel.

Key facts (from the instruction cost model, which is the graded metric):
  - fp8(e4m3) matmuls with MatmulPerfMode.DoubleRow cost 0.5 cycles per
    OUTPUT column and contract 2 k-subtiles per instruction -> 4x the
    fp32r/bf16 rate for a fixed contraction.
  - bf16 and fp16 matmuls cost the same (1.0 cyc/row), so the non-fp8
    pipeline (v, et, out2, Wo) runs in fp16 for 8x less rounding error.
  - ACT exp costs 0.833ns/elem regardless of dtype (+~185ns/inst): exp over
    [128, 2, 512] PSUM groups amortizes it; ACT is exp-only in phase B.
  - DVE fp16 elementwise ops hit the 2x perf mode (~376ns per [128,512]).
  - All DMA transfers serialize on one shared device at ~360GB/s, in issue
    order -- so loads are issued in dependency-criticality order.

Design (per core: one batch element, 4 heads = 512 of 2048 Wq/Wk/Wv cols
and 512 rows of Wo; host sums the 4 tensor-parallel partials per batch):
  - Everything SBUF-resident; no DRAM scratch round-trips.
  - Phase A in three PSUM scopes so no accumulation group is left open
    waiting on late DMA data:
      A1: v-proj chain 1 (x_hi @ Wv_hi, fp8 DR) -> v_sb fp16 (holds 64*v;
          the 64 is divided out for free via ones=64 in the denominator).
      A2: q/k-proj (fp8 DR from x_hi) -> DVE rope-mul (cos+sin factors with
          all fp8 scale folding) -> fp8 q/k; SBUF->SBUF DMAs rearrange
          [128d, T] into the [64, 2, T] d-split DoubleRow layout.
      A3: v-proj correction chains (x_lo @ Wv_hi + x_hi @ Wv_lo, fp8 DR)
          added into v_sb by DVE (compensates fp8 quantization of x and Wv).
  - Phase B per tq-512 block, per head: scores fp8 DR over d=[64,2]; exp on
    ACT (scale=1/32 descales fp8) over 2-bank PSUM groups -> fp16 et;
    attn@v fp16; denominator = DVE/Pool fp16 tree over the 16 et tiles +
    one ones[128,128]@acc matmul (replaces 16 PE matmuls per (h,tq)).
  - out-proj fp16, emitted through a queue that interleaves its (qt,oc)
    groups between attention exp-groups of the NEXT tq block, so PE never
    stalls on the single out-proj PSUM bank; the final block's groups
    alternate between two banks (reusing the idle sp bank).

Numerics: fp8 quantization residuals are compensated where errors pass
straight through (v), and exploited where softmax attenuates them
(q/k/scores: fp8 noise lands on logits ~N(0,0.2^2) as ~1.5% attention
weight noise -> ~1.3e-2 worst-case rel err vs the 2e-2 gate).
"""

import sys

sys.path.insert(0, "/opt/trn_rl_repo")

import math

import numpy as np
import ml_dtypes

import concourse.bass as bass
import concourse.mybir as mybir
import concourse.tile as tile
from concourse import bacc, bass_utils

F32 = mybir.dt.float32
FP16 = mybir.dt.float16
FP8 = mybir.dt.float8e4
NP_FP8 = ml_dtypes.float8_e4m3
Copy = mybir.ActivationFunctionType.Copy
Exp = mybir.ActivationFunctionType.Exp
DR = mybir.MatmulPerfMode.DoubleRow

HIDDEN = 2048
HEADS = 16
HEAD_DIM = 128
SEQ = 2048
BATCH = 2
N_CORES = 8
MP = 4
HG = HEADS // MP  # 4 heads per core
THETA = 10000.0

WSCALE = 64.0  # host premultiplier on Wq/Wk/Wv/Wo before fp8 cast
O2SCALE = 32.0  # fp8 scale on the normalized attention output


def build_attention_nc(use_mask=False):
    T, C, D, DG = SEQ, HIDDEN, HEAD_DIM, HG * HEAD_DIM
    CCH = C // 128  # 16 contraction subtiles
    CCP = CCH // 2  # 8 DoubleRow pairs
    NTK = T // 128  # 16
    NTQ = T // 512  # 4 tq blocks
    NG = NTK // 2  # 8 exp groups per (h, tq)

    nc = bacc.Bacc("TRN2", target_bir_lowering=False, debug=False)

    x8hi = nc.dram_tensor("x8hi", [C, T], FP8, kind="ExternalInput").ap()
    x8lo = nc.dram_tensor("x8lo", [C, T], FP8, kind="ExternalInput").ap()
    wv8hi = nc.dram_tensor("wv8hi", [C, DG], FP8, kind="ExternalInput").ap()
    wv8lo = nc.dram_tensor("wv8lo", [C, DG], FP8, kind="ExternalInput").ap()
    w8q = nc.dram_tensor("w8q", [C, DG], FP8, kind="ExternalInput").ap()
    w8k = nc.dram_tensor("w8k", [C, DG], FP8, kind="ExternalInput").ap()
    wo8hi = nc.dram_tensor("wo8hi", [HG, D, C], FP8, kind="ExternalInput").ap()
    wo8lo = nc.dram_tensor("wo8lo", [HG, D, C], FP8, kind="ExternalInput").ap()
    cfq = nc.dram_tensor("cfq", [D, T], FP16, kind="ExternalInput").ap()
    cfk = nc.dram_tensor("cfk", [D, T], FP16, kind="ExternalInput").ap()
    if use_mask:
        maskT = nc.dram_tensor("maskT", [T, T], F32, kind="ExternalInput").ap()
    out = nc.dram_tensor("out", [T, C], FP16, kind="ExternalOutput").ap()

    with tile.TileContext(nc) as tc:
        with (
            tc.tile_pool(name="res", bufs=1) as res,
            tc.tile_pool(name="accp", bufs=2) as accp,
            tc.tile_pool(name="o2sb", bufs=2) as o2sb,
            tc.tile_pool(name="rtp", bufs=2) as rtp,
            tc.tile_pool(name="otp", bufs=4) as otp,
            tc.tile_pool(name="mkp", bufs=2) as mkp,
        ):
            wo8hi_s = res.tile([128, HG, C], FP8, tag="wo8hi_s")
            wo8lo_s = res.tile([128, HG, C], FP8, tag="wo8lo_s")
            cfq_s = res.tile([128, T], FP16, tag="cfq_s")
            cfk_s = res.tile([128, T], FP16, tag="cfk_s")
            v_sb = res.tile([128, NTK, DG], FP16, tag="v_sb")
            qT16 = res.tile([128, HG, T], FP16, tag="qT16")
            kT16 = res.tile([128, HG, T], FP16, tag="kT16")
            ones = res.tile([128, 128], FP16, tag="ones")

            # ones = WSCALE/O2SCALE: v_sb holds 64*v and the normalize
            # multiply should emit 32*out2 (the fp8 out-proj scale), so the
            # denominator matmul pre-scales by 64/32 = 2.
            nc.vector.memset(ones[:], WSCALE / O2SCALE)

            # ---- phase A (x/w staging SBUF is scoped so phase B can hold
            # a full slot of et tiles) ----
            with (
                tc.tile_pool(name="xw", bufs=1) as xw,
                tc.tile_pool(name="aps", bufs=1, space="PSUM") as aps,
            ):
                x8hi_s = xw.tile([128, CCH, T], FP8, tag="x8hi_s")
                x8lo_s = xw.tile([128, CCH, T], FP8, tag="x8lo_s")
                wv8hi_s = xw.tile([128, CCH, DG], FP8, tag="wv8hi_s")
                wv8lo_s = xw.tile([128, CCH, DG], FP8, tag="wv8lo_s")
                w8q_s = xw.tile([128, CCH, DG], FP8, tag="w8q_s")
                w8k_s = xw.tile([128, CCH, DG], FP8, tag="w8k_s")

                # Input loads: ALL on one queue (SP) because transfers from
                # different queues round-robin on the shared DMA device --
                # one queue enforces global dependency-criticality order.
                # wv8hi chunk-pairs interleave with x8hi pairs so each
                # DoubleRow ccp step's operands arrive together.
                wvh_r = wv8hi.rearrange("(a p) b -> p a b", p=128)
                wvl_r = wv8lo.rearrange("(a p) b -> p a b", p=128)
                for ccp in range(CCP):
                    nc.sync.dma_start(
                        wv8hi_s[:, 2 * ccp : 2 * ccp + 2, :],
                        wvh_r[:, 2 * ccp : 2 * ccp + 2, :],
                    )
                    for cc in (2 * ccp, 2 * ccp + 1):
                        nc.sync.dma_start(
                            x8hi_s[:, cc, :], x8hi[cc * 128 : (cc + 1) * 128, :]
                        )
                nc.sync.dma_start(
                    w8q_s[:, :, :], w8q.rearrange("(a p) b -> p a b", p=128)
                )
                nc.sync.dma_start(
                    w8k_s[:, :, :], w8k.rearrange("(a p) b -> p a b", p=128)
                )
                nc.sync.dma_start(cfq_s[:D, :], cfq)
                nc.sync.dma_start(cfk_s[:D, :], cfk)
                nc.sync.dma_start(wv8lo_s[:, :, :], wvl_r)
                for cc in range(CCH):
                    nc.sync.dma_start(
                        x8lo_s[:, cc, :], x8lo[cc * 128 : (cc + 1) * 128, :]
                    )
                for hh in range(HG):
                    nc.sync.dma_start(wo8hi_s[:D, hh, :], wo8hi[hh])
                    nc.sync.dma_start(wo8lo_s[:D, hh, :], wo8lo[hh])

                # A1/A3 rounds and A2's qk tiles all rotate through one
                # 8-bank tag space -- no PSUM scope transitions in phase A.
                def vchain(round_tks, chains, vop):
                    pvs = [
                        aps.tile([128, DG], F32, tag=f"pv{i}", name=f"pv{i}")
                        for i in range(len(round_tks))
                    ]
                    n = len(chains) * CCP
                    k = 0
                    for ccp in range(CCP):
                        for x_s, w_s in chains:
                            for i, tk in enumerate(round_tks):
                                nc.tensor.matmul(
                                    pvs[i][:],
                                    x_s[:, 2 * ccp : 2 * ccp + 2, tk * 128 : (tk + 1) * 128],
                                    w_s[:, 2 * ccp : 2 * ccp + 2, :],
                                    start=(k == 0),
                                    stop=(k == n - 1),
                                    perf_mode=DR,
                                )
                            k += 1
                    for i, tk in enumerate(round_tks):
                        vop(tk, pvs[i])

                # A1: v-proj chain 1 (x_hi @ Wv_hi) -> v_sb (holds 64*v)
                for r in range(2):
                    vchain(
                        list(range(r * 8, r * 8 + 8)),
                        [(x8hi_s, wv8hi_s)],
                        lambda tk, pv: nc.vector.tensor_copy(v_sb[:, tk, :], pv[:]),
                    )

                # A2: q/k projection -> rope-mul -> fp8 -> d-split rearrange
                qpi = [0]
                for h in range(HG):
                    for wi, (w_s, cf_s, dst) in enumerate(
                        ((w8q_s, cfq_s, qT16), (w8k_s, cfk_s, kT16))
                    ):
                        for tq in range(NTQ):
                            qp = aps.tile(
                                [128, 512], F32, tag=f"pv{qpi[0] % 8}", name="qp"
                            )
                            qpi[0] += 1
                            for ccp in range(CCP):
                                nc.tensor.matmul(
                                    qp[:],
                                    w_s[:, 2 * ccp : 2 * ccp + 2, h * D : (h + 1) * D],
                                    x8hi_s[:, 2 * ccp : 2 * ccp + 2, tq * 512 : (tq + 1) * 512],
                                    start=(ccp == 0),
                                    stop=(ccp == CCP - 1),
                                    perf_mode=DR,
                                )
                            nc.vector.tensor_mul(
                                dst[:D, h, tq * 512 : (tq + 1) * 512],
                                qp[:D, :],
                                cf_s[:D, tq * 512 : (tq + 1) * 512],
                            )

                # A3: v-proj correction chains (x_lo@Wv_hi + x_hi@Wv_lo)
                for r in range(2):
                    vchain(
                        list(range(r * 8, r * 8 + 8)),
                        [(x8lo_s, wv8hi_s), (x8hi_s, wv8lo_s)],
                        lambda tk, pv: nc.vector.tensor_add(
                            v_sb[:, tk, :], v_sb[:, tk, :], pv[:]
                        ),
                    )

            # ---- phase B: slot-pipelined attention + out-projection ----
            # Slot k = (tq, h). PE emits scores(k) interleaved with
            # attn@v(k-1) -- a FULL slot of lag, so attn@v never waits on
            # ACT's exp stream. Finisher(k-1) (denominator matmul, recip,
            # normalize) and out-proj drips fill the remaining PE slack.
            with (
                tc.tile_pool(name="etp", bufs=1) as etp,
                tc.tile_pool(name="scps", bufs=1, space="PSUM") as scps,
                tc.tile_pool(name="o2ps", bufs=1, space="PSUM") as o2ps,
                tc.tile_pool(name="mps", bufs=1, space="PSUM") as mps,
            ):
                og_queue = []
                og_i = [0]
                fin_queue = []
                TAGROT = ["fp", "sp"]

                def emit_og(tags=("fp", "sp")):
                    tag = tags[og_i[0] % len(tags)]
                    og_i[0] += 1
                    qt, oc, o2t, tq = og_queue.pop(0)
                    fp = mps.tile([128, 512], F32, tag=tag, name="fp") if tag in (
                        "fp",
                        "sp",
                    ) else o2ps.tile([128, 512], F32, tag=tag, name="fp")
                    o2h, o2l = o2t
                    chains = ((o2h, wo8hi_s), (o2l, wo8hi_s), (o2h, wo8lo_s))
                    n = len(chains) * 2
                    ci = 0
                    for lhs, rhs in chains:
                        for hp in (0, 2):
                            nc.tensor.matmul(
                                fp[:],
                                lhs[:D, hp : hp + 2, qt * 128 : (qt + 1) * 128],
                                rhs[:D, hp : hp + 2, oc * 512 : (oc + 1) * 512],
                                start=(ci == 0),
                                stop=(ci == n - 1),
                                perf_mode=DR,
                            )
                            ci += 1
                    ot = otp.tile([128, 512], FP16, tag="ot")
                    if og_i[0] % 2 == 0:
                        nc.scalar.activation(
                            ot[:], fp[:], Copy, scale=float(1.0 / (WSCALE * O2SCALE))
                        )
                    else:
                        nc.vector.tensor_scalar_mul(
                            ot[:], fp[:], 1.0 / (WSCALE * O2SCALE)
                        )
                    nc.sync.dma_start(
                        out[
                            tq * 512 + qt * 128 : tq * 512 + (qt + 1) * 128,
                            oc * 512 : (oc + 1) * 512,
                        ],
                        ot[:],
                    )

                def flush_fin():
                    # out-proj groups for a tq block enter the queue HERE,
                    # once its last head is normalized -- dripping them any
                    # earlier would read o2t[h3] before it is written.
                    acc, o2p, o2t, h, tq = fin_queue.pop(0)
                    spp = mps.tile([128, 512], F32, tag="sp", name="spp")
                    nc.tensor.matmul(spp[:], ones[:], acc[:], start=True, stop=True)
                    rt = rtp.tile([128, 512], F32, tag="rt")
                    nc.vector.reciprocal(rt[:], spp[:])
                    m32 = rtp.tile([128, 512], FP16, tag="m32", bufs=2)
                    nc.vector.tensor_mul(m32[:D, :], o2p[:D, :], rt[:D, :])
                    o2h, o2l = o2t
                    nc.vector.tensor_copy(o2h[:D, h, :], m32[:D, :])
                    nc.vector.tensor_tensor(o2l[:D, h, :], m32[:D, :], o2h[:D, h, :], mybir.AluOpType.subtract)
                    if h == HG - 1:
                        for qt in range(4):
                            for oc in range(4):
                                og_queue.append((qt, oc, o2t, tq))

                slots = [(tq, h) for tq in range(NTQ) for h in range(HG)]
                prev = None  # (ets, o2p, o2t, h, acc) of slot k-1

                def emit_av(prev, g):
                    ets, o2p, o2t, h, acc, _tq = prev
                    et = ets[g]
                    for j in range(2):
                        tk = 2 * g + j
                        nc.tensor.matmul(
                            o2p[:],
                            v_sb[:, tk, h * D : (h + 1) * D],
                            et[:, j, :],
                            start=(tk == 0),
                            stop=(tk == NTK - 1),
                        )
                    eng = nc.vector if g % 2 == 0 else nc.gpsimd
                    eng.tensor_add(et[:, 0, :], et[:, 0, :], et[:, 1, :])
                    if g == 1:
                        nc.vector.tensor_add(acc[:], ets[0][:, 0, :], ets[1][:, 0, :])
                    elif g > 1:
                        nc.vector.tensor_add(acc[:], acc[:], et[:, 0, :])

                for k, (tq, h) in enumerate(slots):
                    if h == 0:
                        o2t = (
                            o2sb.tile([128, HG, 512], FP8, tag="o2h", name="o2h"),
                            o2sb.tile([128, HG, 512], FP8, tag="o2l", name="o2l"),
                        )
                    o2p = o2ps.tile([128, 512], F32, tag=f"o2p{k % 2}", name="o2p")
                    acc = accp.tile([128, 512], FP16, tag="acc")
                    ets = []
                    for g in range(NG):
                        sc = scps.tile(
                            [128, 2, 512], F32, tag=f"sc{g % 2}", name="sc"
                        )
                        for j in range(2):
                            tk = 2 * g + j
                            nc.tensor.matmul(
                                sc[:, j, :],
                                kT16[:D, h, tk * 128 : (tk + 1) * 128],
                                qT16[:D, h, tq * 512 : (tq + 1) * 512],
                                start=True,
                                stop=True,
                            )
                        if use_mask:
                            mt = mkp.tile([128, 2, 512], F32, tag="mt")
                            for j in range(2):
                                nc.gpsimd.dma_start(
                                    mt[:, j, :],
                                    maskT[
                                        (2 * g + j) * 128 : (2 * g + j + 1) * 128,
                                        tq * 512 : (tq + 1) * 512,
                                    ],
                                )
                            nc.vector.tensor_add(sc[:, :, :], sc[:, :, :], mt[:])
                        et = etp.tile([128, 2, 512], FP16, tag=f"et{(k % 2) * 8 + g}")
                        nc.scalar.activation(et[:], sc[:], Exp)
                        ets.append(et)
                        # previous slot's attn@v + denominator tree
                        if prev is not None:
                            emit_av(prev, g)
                        if g == 2 and fin_queue:
                            flush_fin()
                        if g in (3, 5, 7) and og_queue:
                            emit_og()
                            if g == 7 and og_queue and len(og_queue) % 4 == 1:
                                emit_og()
                    if prev is not None:
                        fin_queue.append((prev[4], prev[1], prev[2], prev[3], prev[5]))
                    prev = (ets, o2p, o2t, h, acc, tq)
                # drain: last slot's attn@v, remaining finishers, then the
                # final out-proj groups rotating over four idle banks
                for g in range(NG):
                    emit_av(prev, g)
                    if g in (3, 5, 7) and og_queue:
                        emit_og()
                fin_queue.append((prev[4], prev[1], prev[2], prev[3], prev[5]))
                while fin_queue:
                    flush_fin()
                og_i[0] = 0
                while og_queue:
                    emit_og(("fp", "o2p0", "sp", "o2p1"))

    nc.compile()
    return nc


def compute_cfacs():
    T, D = SEQ, HEAD_DIM
    freq = 1.0 / THETA ** (np.arange(0, D, 2, dtype=np.float64) / D)
    t = np.arange(T, dtype=np.float64)
    m = np.einsum("i,j->ij", t, freq)
    m = np.concatenate([m, m], axis=-1)
    cfac = (np.cos(m) + np.sin(m)).T  # [D, T]
    cfq = (cfac / math.sqrt(T) / WSCALE).astype(np.float16)
    cfk = (cfac / WSCALE).astype(np.float16)
    return cfq, cfk


_NC_CACHE = {}


def _get_nc(use_mask):
    key = bool(use_mask)
    if key not in _NC_CACHE:
        _NC_CACHE[key] = build_attention_nc(use_mask=key)
    return _NC_CACHE[key]


def _split_fp8(a):
    hi = a.astype(NP_FP8)
    lo = (a - hi.astype(np.float32)).astype(NP_FP8)
    return hi, lo


def _make_in_maps(input_ids, Wq, Wk, Wv, Wo, attention_mask=None):
    DG = HG * HEAD_DIM
    cfq, cfk = compute_cfacs()
    xhi, xlo = [], []
    for bi in range(BATCH):
        xT = np.ascontiguousarray(input_ids[bi].T)
        hi, lo = _split_fp8(xT)
        xhi.append(hi)
        xlo.append(lo)
    in_maps = []
    for core in range(N_CORES):
        bi, g = divmod(core, MP)
        sl = slice(g * DG, (g + 1) * DG)
        wvhi, wvlo = _split_fp8(np.ascontiguousarray(Wv[:, sl]) * WSCALE)
        wohi, wolo = _split_fp8(
            np.ascontiguousarray(Wo[sl, :]).reshape(HG, HEAD_DIM, HIDDEN) * WSCALE
        )
        m = {
            "x8hi": xhi[bi],
            "x8lo": xlo[bi],
            "wv8hi": wvhi,
            "wv8lo": wvlo,
            "w8q": (np.ascontiguousarray(Wq[:, sl]) * WSCALE).astype(NP_FP8),
            "w8k": (np.ascontiguousarray(Wk[:, sl]) * WSCALE).astype(NP_FP8),
            "wo8hi": wohi,
            "wo8lo": wolo,
            "cfq": cfq,
            "cfk": cfk,
        }
        if attention_mask is not None:
            m["maskT"] = np.ascontiguousarray(attention_mask[bi, 0].T).astype(
                np.float32
            )
        in_maps.append(m)
    return in_maps


def prepare_for_bench(inputs):
    input_ids = np.asarray(inputs["input_ids"], dtype=np.float32)
    Wq = np.asarray(inputs["Wq"], dtype=np.float32)
    Wk = np.asarray(inputs["Wk"], dtype=np.float32)
    Wv = np.asarray(inputs["Wv"], dtype=np.float32)
    Wo = np.asarray(inputs["Wo"], dtype=np.float32)
    return _get_nc(False), _make_in_maps(input_ids, Wq, Wk, Wv, Wo)


def kernel(input_ids, attention_mask, Wq, Wk, Wv, Wo):
    input_ids = np.asarray(input_ids, dtype=np.float32)
    attention_mask = np.asarray(attention_mask, dtype=np.float32)
    Wq = np.asarray(Wq, dtype=np.float32)
    Wk = np.asarray(Wk, dtype=np.float32)
    Wv = np.asarray(Wv, dtype=np.float32)
    Wo = np.asarray(Wo, dtype=np.float32)

    b, t, c = input_ids.shape
    assert (b, t, c) == (BATCH, SEQ, HIDDEN)

    use_mask = bool(np.any(attention_mask))
    nc = _get_nc(use_mask)
    in_maps = _make_in_maps(
        input_ids, Wq, Wk, Wv, Wo, attention_mask if use_mask else None
    )

    res = bass_utils.run_bass_kernel_spmd(nc, in_maps, core_ids=list(range(N_CORES)))

    out = np.zeros((BATCH, SEQ, HIDDEN), dtype=np.float32)
    for bi in range(BATCH):
        acc = res.results[bi * MP]["out"].astype(np.float32)
        for g in range(1, MP):
            acc = acc + res.results[bi * MP + g]["out"].astype(np.float32)
        out[bi] = acc
    return out


# revision 18
# speedup vs baseline: 1.6861x; 1.0143x over previous
"""v17: fp8-DoubleRow rewrite of the LGeM self-attention kernel.

Key facts (from the instruction cost model, which is the graded metric):
  - fp8(e4m3) matmuls with MatmulPerfMode.DoubleRow cost 0.5 cycles per
    OUTPUT column and contract 2 k-subtiles per instruction -> 4x the
    fp32r/bf16 rate for a fixed contraction.
  - bf16 and fp16 matmuls cost the same (1.0 cyc/row), so the non-fp8
    pipeline (v, et, out2, Wo) runs in fp16 for 8x less rounding error.
  - ACT exp costs 0.833ns/elem regardless of dtype (+~185ns/inst): exp over
    [128, 2, 512] PSUM groups amortizes it; ACT is exp-only in phase B.
  - DVE fp16 elementwise ops hit the 2x perf mode (~376ns per [128,512]).
  - All DMA transfers serialize on one shared device at ~360GB/s, in issue
    order -- so loads are issued in dependency-criticality order.

Design (per core: one batch element, 4 heads = 512 of 2048 Wq/Wk/Wv cols
and 512 rows of Wo; host sums the 4 tensor-parallel partials per batch):
  - Everything SBUF-resident; no DRAM scratch round-trips.
  - Phase A in three PSUM scopes so no accumulation group is left open
    waiting on late DMA data:
      A1: v-proj chain 1 (x_hi @ Wv_hi, fp8 DR) -> v_sb fp16 (holds 64*v;
          the 64 is divided out for free via ones=64 in the denominator).
      A2: q/k-proj (fp8 DR from x_hi) -> DVE rope-mul (cos+sin factors with
          all fp8 scale folding) -> fp8 q/k; SBUF->SBUF DMAs rearrange
          [128d, T] into the [64, 2, T] d-split DoubleRow layout.
      A3: v-proj correction chains (x_lo @ Wv_hi + x_hi @ Wv_lo, fp8 DR)
          added into v_sb by DVE (compensates fp8 quantization of x and Wv).
  - Phase B per tq-512 block, per head: scores fp8 DR over d=[64,2]; exp on
    ACT (scale=1/32 descales fp8) over 2-bank PSUM groups -> fp16 et;
    attn@v fp16; denominator = DVE/Pool fp16 tree over the 16 et tiles +
    one ones[128,128]@acc matmul (replaces 16 PE matmuls per (h,tq)).
  - out-proj fp16, emitted through a queue that interleaves its (qt,oc)
    groups between attention exp-groups of the NEXT tq block, so PE never
    stalls on the single out-proj PSUM bank; the final block's groups
    alternate between two banks (reusing the idle sp bank).

Numerics: fp8 quantization residuals are compensated where errors pass
straight through (v), and exploited where softmax attenuates them
(q/k/scores: fp8 noise lands on logits ~N(0,0.2^2) as ~1.5% attention
weight noise -> ~1.3e-2 worst-case rel err vs the 2e-2 gate).
"""

import sys

sys.path.insert(0, "/opt/trn_rl_repo")

import math

import numpy as np
import ml_dtypes

import concourse.bass as bass
import concourse.mybir as mybir
import concourse.tile as tile
from concourse import bacc, bass_utils

F32 = mybir.dt.float32
FP16 = mybir.dt.float16
FP8 = mybir.dt.float8e4
NP_FP8 = ml_dtypes.float8_e4m3
Copy = mybir.ActivationFunctionType.Copy
Exp = mybir.ActivationFunctionType.Exp
DR = mybir.MatmulPerfMode.DoubleRow

HIDDEN = 2048
HEADS = 16
HEAD_DIM = 128
SEQ = 2048
BATCH = 2
N_CORES = 8
MP = 4
HG = HEADS // MP  # 4 heads per core
THETA = 10000.0

WSCALE = 64.0  # host premultiplier on Wq/Wk/Wv/Wo before fp8 cast
O2SCALE = 32.0  # fp8 scale on the normalized attention output


def build_attention_nc(use_mask=False):
    T, C, D, DG = SEQ, HIDDEN, HEAD_DIM, HG * HEAD_DIM
    CCH = C // 128  # 16 contraction subtiles
    CCP = CCH // 2  # 8 DoubleRow pairs
    NTK = T // 128  # 16
    NTQ = T // 512  # 4 tq blocks
    NG = NTK // 2  # 8 exp groups per (h, tq)

    nc = bacc.Bacc("TRN2", target_bir_lowering=False, debug=False)

    x8hi = nc.dram_tensor("x8hi", [C, T], FP8, kind="ExternalInput").ap()
    x8lo = nc.dram_tensor("x8lo", [C, T], FP8, kind="ExternalInput").ap()
    wv8hi = nc.dram_tensor("wv8hi", [C, DG], FP8, kind="ExternalInput").ap()
    wv8lo = nc.dram_tensor("wv8lo", [C, DG], FP8, kind="ExternalInput").ap()
    w8q = nc.dram_tensor("w8q", [C, DG], FP8, kind="ExternalInput").ap()
    w8k = nc.dram_tensor("w8k", [C, DG], FP8, kind="ExternalInput").ap()
    wo8hi = nc.dram_tensor("wo8hi", [HG, D, C], FP8, kind="ExternalInput").ap()
    wo8lo = nc.dram_tensor("wo8lo", [HG, D, C], FP8, kind="ExternalInput").ap()
    cfq = nc.dram_tensor("cfq", [D, T], FP16, kind="ExternalInput").ap()
    cfk = nc.dram_tensor("cfk", [D, T], FP16, kind="ExternalInput").ap()
    if use_mask:
        maskT = nc.dram_tensor("maskT", [T, T], F32, kind="ExternalInput").ap()
    out = nc.dram_tensor("out", [T, C], FP16, kind="ExternalOutput").ap()

    with tile.TileContext(nc) as tc:
        with (
            tc.tile_pool(name="res", bufs=1) as res,
            tc.tile_pool(name="accp", bufs=2) as accp,
            tc.tile_pool(name="o2sb", bufs=2) as o2sb,
            tc.tile_pool(name="rtp", bufs=2) as rtp,
            tc.tile_pool(name="otp", bufs=4) as otp,
            tc.tile_pool(name="mkp", bufs=2) as mkp,
        ):
            wo8hi_s = res.tile([128, HG, C], FP8, tag="wo8hi_s")
            wo8lo_s = res.tile([128, HG, C], FP8, tag="wo8lo_s")
            cfq_s = res.tile([128, T], FP16, tag="cfq_s")
            cfk_s = res.tile([128, T], FP16, tag="cfk_s")
            v_sb = res.tile([128, NTK, DG], FP16, tag="v_sb")
            qT16 = res.tile([128, HG, T], FP16, tag="qT16")
            kT16 = res.tile([128, HG, T], FP16, tag="kT16")
            ones = res.tile([128, 128], FP16, tag="ones")

            # ones = WSCALE/O2SCALE: v_sb holds 64*v and the normalize
            # multiply should emit 32*out2 (the fp8 out-proj scale), so the
            # denominator matmul pre-scales by 64/32 = 2.
            nc.vector.memset(ones[:], WSCALE / O2SCALE)

            # ---- phase A (x/w staging SBUF is scoped so phase B can hold
            # a full slot of et tiles) ----
            _ps_cm = tc.tile_pool(name="ps", bufs=1, space="PSUM")
            ps = _ps_cm.__enter__()

            def ps_pair(tag):
                return ps.tile([128, 2, 512], F32, tag=tag, name=tag)

            def ps_one(tag):
                return ps.tile([128, 512], F32, tag=tag, name=tag)

            with tc.tile_pool(name="xw", bufs=1) as xw:
                x8hi_s = xw.tile([128, CCH, T], FP8, tag="x8hi_s")
                x8lo_s = xw.tile([128, CCH, T], FP8, tag="x8lo_s")
                wv8hi_s = xw.tile([128, CCH, DG], FP8, tag="wv8hi_s")
                wv8lo_s = xw.tile([128, CCH, DG], FP8, tag="wv8lo_s")
                w8q_s = xw.tile([128, CCH, DG], FP8, tag="w8q_s")
                w8k_s = xw.tile([128, CCH, DG], FP8, tag="w8k_s")

                # Input loads: ALL on one queue (SP) because transfers from
                # different queues round-robin on the shared DMA device --
                # one queue enforces global dependency-criticality order.
                # wv8hi chunk-pairs interleave with x8hi pairs so each
                # DoubleRow ccp step's operands arrive together.
                wvh_r = wv8hi.rearrange("(a p) b -> p a b", p=128)
                wvl_r = wv8lo.rearrange("(a p) b -> p a b", p=128)
                for ccp in range(CCP):
                    nc.sync.dma_start(
                        wv8hi_s[:, 2 * ccp : 2 * ccp + 2, :],
                        wvh_r[:, 2 * ccp : 2 * ccp + 2, :],
                    )
                    for cc in (2 * ccp, 2 * ccp + 1):
                        nc.sync.dma_start(
                            x8hi_s[:, cc, :], x8hi[cc * 128 : (cc + 1) * 128, :]
                        )
                nc.sync.dma_start(
                    w8q_s[:, :, :], w8q.rearrange("(a p) b -> p a b", p=128)
                )
                nc.sync.dma_start(
                    w8k_s[:, :, :], w8k.rearrange("(a p) b -> p a b", p=128)
                )
                nc.sync.dma_start(cfq_s[:D, :], cfq)
                nc.sync.dma_start(cfk_s[:D, :], cfk)
                nc.sync.dma_start(wv8lo_s[:, :, :], wvl_r)
                for cc in range(CCH):
                    nc.sync.dma_start(
                        x8lo_s[:, cc, :], x8lo[cc * 128 : (cc + 1) * 128, :]
                    )
                for hh in range(HG):
                    nc.sync.dma_start(wo8hi_s[:D, hh, :], wo8hi[hh])
                    nc.sync.dma_start(wo8lo_s[:D, hh, :], wo8lo[hh])

                # A1/A3 rounds and A2's qk tiles all rotate through one
                # 8-bank tag space -- no PSUM scope transitions in phase A.
                def alloc_accs(n):
                    accs = []
                    if n > 0:
                        w = ps_pair("w0")
                        accs += [w[:, 0, :], w[:, 1, :]]
                    if n > 2:
                        w = ps_pair("w1")
                        accs += [w[:, 0, :], w[:, 1, :]]
                    for tag in ("o2p0", "o2p1", "sp", "fp")[: max(0, n - 4)]:
                        accs.append(ps_one(tag)[:, :])
                    return accs[:n]

                def alloc_sing(n):
                    return [
                        ps_one(tag)[:, :]
                        for tag in ("o2p0", "o2p1", "sp", "fp")[:n]
                    ]

                def vchain(round_tks, chains, vop, accs):
                    pvs = accs
                    n = len(chains) * CCP
                    k = 0
                    for ccp in range(CCP):
                        for x_s, w_s in chains:
                            for i, tk in enumerate(round_tks):
                                nc.tensor.matmul(
                                    pvs[i][:],
                                    x_s[:, 2 * ccp : 2 * ccp + 2, tk * 128 : (tk + 1) * 128],
                                    w_s[:, 2 * ccp : 2 * ccp + 2, :],
                                    start=(k == 0),
                                    stop=(k == n - 1),
                                    perf_mode=DR,
                                )
                            k += 1
                    for i, tk in enumerate(round_tks):
                        vop(tk, pvs[i])

                # A1: v-proj chain 1 (x_hi @ Wv_hi) -> v_sb (holds 64*v);
                # PSUM->SBUF copies alternate DVE/ACT
                def a1_copy(tk, pv):
                    if tk % 2 == 0:
                        nc.vector.tensor_copy(v_sb[:, tk, :], pv[:])
                    else:
                        nc.scalar.activation(v_sb[:, tk, :], pv[:], Copy)

                for r in range(2):
                    vchain(
                        list(range(r * 8, r * 8 + 8)),
                        [(x8hi_s, wv8hi_s)],
                        a1_copy,
                        alloc_accs(8),
                    )

                # A2: q/k projection -> rope-mul -> fp8 -> d-split rearrange
                qpi = [0]
                qk_accs = [None]
                for h in range(HG):
                    for wi, (w_s, cf_s, dst) in enumerate(
                        ((w8q_s, cfq_s, qT16), (w8k_s, cfk_s, kT16))
                    ):
                        for tq in range(NTQ):
                            if qpi[0] % 8 == 0:
                                qk_accs[0] = alloc_accs(8)
                            qp = qk_accs[0][qpi[0] % 8]
                            qpi[0] += 1
                            for ccp in range(CCP):
                                nc.tensor.matmul(
                                    qp[:],
                                    w_s[:, 2 * ccp : 2 * ccp + 2, h * D : (h + 1) * D],
                                    x8hi_s[:, 2 * ccp : 2 * ccp + 2, tq * 512 : (tq + 1) * 512],
                                    start=(ccp == 0),
                                    stop=(ccp == CCP - 1),
                                    perf_mode=DR,
                                )
                            nc.vector.tensor_mul(
                                dst[:D, h, tq * 512 : (tq + 1) * 512],
                                qp[:D, :],
                                cf_s[:D, tq * 512 : (tq + 1) * 512],
                            )

                # A3: v-proj correction chains (x_lo@Wv_hi + x_hi@Wv_lo);
                # the last round uses the single-bank tags so phase B's
                # score banks (w0/w1) are free as early as possible
                a3_add = lambda tk, pv: nc.vector.tensor_add(
                    v_sb[:, tk, :], v_sb[:, tk, :], pv[:]
                )
                a3_chains = [(x8lo_s, wv8hi_s), (x8hi_s, wv8lo_s)]
                vchain(list(range(0, 8)), a3_chains, a3_add, alloc_accs(8))
                vchain(list(range(8, 12)), a3_chains, a3_add, alloc_accs(4))
                vchain(list(range(12, 16)), a3_chains, a3_add, alloc_sing(4))

            # ---- phase B: slot-pipelined attention + out-projection ----
            # Slot k = (tq, h). PE emits scores(k) interleaved with
            # attn@v(k-1) -- a FULL slot of lag, so attn@v never waits on
            # ACT's exp stream. Finisher(k-1) (denominator matmul, recip,
            # normalize) and out-proj drips fill the remaining PE slack.
            with tc.tile_pool(name="etp", bufs=1) as etp:
                og_queue = []
                og_i = [0]
                fin_queue = []
                TAGROT = ["fp", "sp"]

                def emit_og(tags=("fp", "sp")):
                    tag = tags[og_i[0] % len(tags)]
                    og_i[0] += 1
                    qt, oc, o2t, tq = og_queue.pop(0)
                    fp = (
                        ps_one(tag) if tag in ("fp", "sp", "o2p0", "o2p1")
                        else ps_pair(tag[:2])[:, int(tag[3]), :]
                    )
                    o2h, o2l = o2t
                    chains = ((o2h, wo8hi_s), (o2l, wo8hi_s), (o2h, wo8lo_s))
                    n = len(chains) * 2
                    ci = 0
                    for lhs, rhs in chains:
                        for hp in (0, 2):
                            nc.tensor.matmul(
                                fp[:],
                                lhs[:D, hp : hp + 2, qt * 128 : (qt + 1) * 128],
                                rhs[:D, hp : hp + 2, oc * 512 : (oc + 1) * 512],
                                start=(ci == 0),
                                stop=(ci == n - 1),
                                perf_mode=DR,
                            )
                            ci += 1
                    ot = otp.tile([128, 512], FP16, tag="ot")
                    if og_i[0] % 2 == 0:
                        nc.scalar.activation(
                            ot[:], fp[:], Copy, scale=float(1.0 / (WSCALE * O2SCALE))
                        )
                    else:
                        nc.vector.tensor_scalar_mul(
                            ot[:], fp[:], 1.0 / (WSCALE * O2SCALE)
                        )
                    nc.sync.dma_start(
                        out[
                            tq * 512 + qt * 128 : tq * 512 + (qt + 1) * 128,
                            oc * 512 : (oc + 1) * 512,
                        ],
                        ot[:],
                    )

                def flush_fin():
                    # out-proj groups for a tq block enter the queue HERE,
                    # once its last head is normalized -- dripping them any
                    # earlier would read o2t[h3] before it is written.
                    acc, o2p, o2t, h, tq = fin_queue.pop(0)
                    spp = ps_one("sp")
                    nc.tensor.matmul(spp[:], ones[:], acc[:], start=True, stop=True)
                    rt = rtp.tile([128, 512], F32, tag="rt")
                    nc.vector.reciprocal(rt[:], spp[:])
                    m32 = rtp.tile([128, 512], FP16, tag="m32", bufs=2)
                    nc.vector.tensor_mul(m32[:D, :], o2p[:D, :], rt[:D, :])
                    o2h, o2l = o2t
                    nc.vector.tensor_copy(o2h[:D, h, :], m32[:D, :])
                    nc.vector.tensor_tensor(o2l[:D, h, :], m32[:D, :], o2h[:D, h, :], mybir.AluOpType.subtract)
                    if h == HG - 1:
                        for qt in range(4):
                            for oc in range(4):
                                og_queue.append((qt, oc, o2t, tq))

                slots = [(tq, h) for tq in range(NTQ) for h in range(HG)]
                prev = None  # (ets, o2p, o2t, h, acc) of slot k-1

                def emit_av(prev, g):
                    ets, o2p, o2t, h, acc, _tq = prev
                    et = ets[g]
                    for j in range(2):
                        tk = 2 * g + j
                        nc.tensor.matmul(
                            o2p[:],
                            v_sb[:, tk, h * D : (h + 1) * D],
                            et[:, j, :],
                            start=(tk == 0),
                            stop=(tk == NTK - 1),
                        )
                    eng = nc.vector if g % 2 == 0 else nc.gpsimd
                    eng.tensor_add(et[:, 0, :], et[:, 0, :], et[:, 1, :])
                    if g == 1:
                        nc.vector.tensor_add(acc[:], ets[0][:, 0, :], ets[1][:, 0, :])
                    elif g > 1:
                        nc.vector.tensor_add(acc[:], acc[:], et[:, 0, :])

                for k, (tq, h) in enumerate(slots):
                    if h == 0:
                        o2t = (
                            o2sb.tile([128, HG, 512], FP8, tag="o2h", name="o2h"),
                            o2sb.tile([128, HG, 512], FP8, tag="o2l", name="o2l"),
                        )
                    o2p = ps_one(f"o2p{k % 2}")
                    acc = accp.tile([128, 512], FP16, tag="acc")
                    ets = []
                    for g in range(NG):
                        sc = ps_pair(f"w{g % 2}")
                        for j in range(2):
                            tk = 2 * g + j
                            nc.tensor.matmul(
                                sc[:, j, :],
                                kT16[:D, h, tk * 128 : (tk + 1) * 128],
                                qT16[:D, h, tq * 512 : (tq + 1) * 512],
                                start=True,
                                stop=True,
                            )
                        if use_mask:
                            mt = mkp.tile([128, 2, 512], F32, tag="mt")
                            for j in range(2):
                                nc.gpsimd.dma_start(
                                    mt[:, j, :],
                                    maskT[
                                        (2 * g + j) * 128 : (2 * g + j + 1) * 128,
                                        tq * 512 : (tq + 1) * 512,
                                    ],
                                )
                            nc.vector.tensor_add(sc[:, :, :], sc[:, :, :], mt[:])
                        et = etp.tile([128, 2, 512], FP16, tag=f"et{(k % 2) * 8 + g}")
                        nc.scalar.activation(et[:], sc[:], Exp)
                        ets.append(et)
                        # previous slot's attn@v + denominator tree
                        if prev is not None:
                            emit_av(prev, g)
                        if g == 2 and fin_queue:
                            flush_fin()
                        if g in (3, 5, 7) and og_queue:
                            emit_og()
                            if g == 7 and og_queue and len(og_queue) % 4 == 1:
                                emit_og()
                    if prev is not None:
                        fin_queue.append((prev[4], prev[1], prev[2], prev[3], prev[5]))
                    prev = (ets, o2p, o2t, h, acc, tq)
                # drain: last slot's attn@v, remaining finishers, then the
                # final out-proj groups rotating over four idle banks
                for g in range(NG):
                    emit_av(prev, g)
                    if g in (3, 5, 7) and og_queue:
                        emit_og()
                fin_queue.append((prev[4], prev[1], prev[2], prev[3], prev[5]))
                while fin_queue:
                    flush_fin()
                og_i[0] = 0
                while og_queue:
                    emit_og(
                        ("fp", "w0_0", "sp", "w1_0", "o2p0", "w0_1", "o2p1", "w1_1")
                    )

            _ps_cm.__exit__(None, None, None)

    nc.compile()
    return nc


def compute_cfacs():
    T, D = SEQ, HEAD_DIM
    freq = 1.0 / THETA ** (np.arange(0, D, 2, dtype=np.float64) / D)
    t = np.arange(T, dtype=np.float64)
    m = np.einsum("i,j->ij", t, freq)
    m = np.concatenate([m, m], axis=-1)
    cfac = (np.cos(m) + np.sin(m)).T  # [D, T]
    cfq = (cfac / math.sqrt(T) / WSCALE).astype(np.float16)
    cfk = (cfac / WSCALE).astype(np.float16)
    return cfq, cfk


_NC_CACHE = {}


def _get_nc(use_mask):
    key = bool(use_mask)
    if key not in _NC_CACHE:
        _NC_CACHE[key] = build_attention_nc(use_mask=key)
    return _NC_CACHE[key]


def _split_fp8(a):
    hi = a.astype(NP_FP8)
    lo = (a - hi.astype(np.float32)).astype(NP_FP8)
    return hi, lo


def _make_in_maps(input_ids, Wq, Wk, Wv, Wo, attention_mask=None):
    DG = HG * HEAD_DIM
    cfq, cfk = compute_cfacs()
    xhi, xlo = [], []
    for bi in range(BATCH):
        xT = np.ascontiguousarray(input_ids[bi].T)
        hi, lo = _split_fp8(xT)
        xhi.append(hi)
        xlo.append(lo)
    in_maps = []
    for core in range(N_CORES):
        bi, g = divmod(core, MP)
        sl = slice(g * DG, (g + 1) * DG)
        wvhi, wvlo = _split_fp8(np.ascontiguousarray(Wv[:, sl]) * WSCALE)
        wohi, wolo = _split_fp8(
            np.ascontiguousarray(Wo[sl, :]).reshape(HG, HEAD_DIM, HIDDEN) * WSCALE
        )
        m = {
            "x8hi": xhi[bi],
            "x8lo": xlo[bi],
            "wv8hi": wvhi,
            "wv8lo": wvlo,
            "w8q": (np.ascontiguousarray(Wq[:, sl]) * WSCALE).astype(NP_FP8),
            "w8k": (np.ascontiguousarray(Wk[:, sl]) * WSCALE).astype(NP_FP8),
            "wo8hi": wohi,
            "wo8lo": wolo,
            "cfq": cfq,
            "cfk": cfk,
        }
        if attention_mask is not None:
            m["maskT"] = np.ascontiguousarray(attention_mask[bi, 0].T).astype(
                np.float32
            )
        in_maps.append(m)
    return in_maps


def prepare_for_bench(inputs):
    input_ids = np.asarray(inputs["input_ids"], dtype=np.float32)
    Wq = np.asarray(inputs["Wq"], dtype=np.float32)
    Wk = np.asarray(inputs["Wk"], dtype=np.float32)
    Wv = np.asarray(inputs["Wv"], dtype=np.float32)
    Wo = np.asarray(inputs["Wo"], dtype=np.float32)
    return _get_nc(False), _make_in_maps(input_ids, Wq, Wk, Wv, Wo)


def kernel(input_ids, attention_mask, Wq, Wk, Wv, Wo):
    input_ids = np.asarray(input_ids, dtype=np.float32)
    attention_mask = np.asarray(attention_mask, dtype=np.float32)
    Wq = np.asarray(Wq, dtype=np.float32)
    Wk = np.asarray(Wk, dtype=np.float32)
    Wv = np.asarray(Wv, dtype=np.float32)
    Wo = np.asarray(Wo, dtype=np.float32)

    b, t, c = input_ids.shape
    assert (b, t, c) == (BATCH, SEQ, HIDDEN)

    use_mask = bool(np.any(attention_mask))
    nc = _get_nc(use_mask)
    in_maps = _make_in_maps(
        input_ids, Wq, Wk, Wv, Wo, attention_mask if use_mask else None
    )

    res = bass_utils.run_bass_kernel_spmd(nc, in_maps, core_ids=list(range(N_CORES)))

    out = np.zeros((BATCH, SEQ, HIDDEN), dtype=np.float32)
    for bi in range(BATCH):
        acc = res.results[bi * MP]["out"].astype(np.float32)
        for g in range(1, MP):
            acc = acc + res.results[bi * MP + g]["out"].astype(np.float32)
        out[bi] = acc
    return out
